# revision 41
# baseline (speedup 1.0000x reference)
"""Multi-head attention kernel for Trainium2, sharded over 8 NeuronCores.

Sharding: data parallel over batch (B=2 -> 4 cores each) x tensor parallel
over heads (12 heads -> 3 heads per core). Each core computes QKV projections,
attention, and a partial output projection for its 3 heads; the per-head
partial output projections are summed on the host (the all-reduce of the
tensor-parallel hint, done during the gather step) and the output bias added.

Layout/precision choices (per core):
  - x and the QKV weights travel in fp16 (halves the input DMA); the QKV
    projection matmuls run fp16 (same PE rate as f32r), psum accumulation
    stays fp32, and q/k are evacuated to f32r tiles so the scores matmul is
    full precision.
  - scores are computed transposed (scoresT [s_k, s_q]) so softmax
    probabilities feed probs@V with no transpose; denominators ride as ones
    columns in the V operand (rows 64..127 of the ctx psum accumulate the
    per-query sums of exp).
  - EDGE mode: probs are written as fp8e4 (exp(z - 2) keeps the range in
    fp8), V is stored as an fp8 pair [v8 | delta_v8] and probs@V runs as a
    DoubleRow fp8 matmul with the probs operand broadcast across the two
    halves: ctx = sum p8*(v8+dv8) -- v at ~fp16 accuracy, probs at fp8, and
    probs@V runs at 0.5 cycles/row: 4x fewer PE cycles than the fp16 path.
  - out projection is f32r; out psum is evacuated on the gpsimd engine
    (DVE handles q/k/v evac + softmax normalize) and DMAs alternate between
    the SP and Pool queues.
"""

from collections import deque

import numpy as np

import concourse.mybir as mybir
from concourse import bacc
from concourse.tile import TileContext
from concourse.bass_utils import run_bass_kernel_spmd

H, D, DH = 12, 768, 64
B, S = 2, 2048
NCORES = 8
CORES_PER_BATCH = 4
HPC = 3  # heads per core
SQ = 512  # query-chunk width
NSQ = S // SQ  # 4
NSK = S // 128  # 16 key chunks
NDC = D // 128  # 6 contraction chunks
NDH = D // 256  # 3 DoubleRow contraction chunks

EDGE = True  # fp8 probs + [v8|dv8] DoubleRow probs@V

F32 = mybir.dt.float32
F32R = mybir.dt.float32r
F16 = mybir.dt.float16
F8 = mybir.dt.float8e4
BF16 = mybir.dt.bfloat16
ADD = mybir.AluOpType.add
SUB = mybir.AluOpType.subtract
MULT = mybir.AluOpType.mult
EXP = mybir.ActivationFunctionType.Exp
DR = mybir.MatmulPerfMode.DoubleRow
SHIFT = -2.0  # probs = exp(z + SHIFT); cancels in normalization
WSCALE = 64.0  # qkv weights are scaled x64 before fp8 quantization (their
# natural magnitude ~0.036 sits in fp8e4m3's subnormal range); the psum is
# scaled back by 1/64 during evacuation


def _build_module():
    nc = bacc.Bacc("TRN2", target_bir_lowering=False, debug=False, num_devices=NCORES)
    # x and the qkv weights travel as fp8 value+residual pairs; the
    # projections run as DoubleRow chains x8*W8 + x8*dW8 + dx8*W8 (the
    # dropped dx*dW term is ~0.1%), halving the projection PE time vs fp16
    xp = nc.declare_dram_parameter("xp", [128, NDH, 2, 2, S], F8, isOutput=False)
    wqk = nc.declare_dram_parameter("wqk", [128, HPC, NDH, 2, 2, 128], F8, isOutput=False)
    wv = nc.declare_dram_parameter("wv", [128, NDH, 2, 2, 256], F8, isOutput=False)
    wo01 = nc.declare_dram_parameter("wo01", [128, D], F32R, isOutput=False)
    wo2 = nc.declare_dram_parameter("wo2", [64, D], F32R, isOutput=False)
    bqk = nc.declare_dram_parameter("bqk", [128, HPC], F32, isOutput=False)
    out = nc.declare_dram_parameter("out", [S, D], BF16, isOutput=True)

    with TileContext(nc) as tc:
        _body(nc, tc, xp, wqk, wv, wo01, wo2, bqk, out)
    nc.compile()
    return nc


def _body(nc, tc, xp, wqk, wv, wo01, wo2, bqk, out):
    with (
        tc.tile_pool(name="persist", bufs=1) as P1,
        tc.tile_pool(name="work", bufs=4) as W2,
        tc.tile_pool(name="probs", bufs=2) as PR,
        # PSUM: 8 banks of [128, 2KB]:
        #   SPS: 2 x [128, 1024] score tiles (4 banks)
        #   CPS: 2 x [128, 512] ctx accumulators (2 banks)
        #   ACC: 2 x [128, <=512] qk / v-pair / out-proj rotation (2 banks)
        tc.tile_pool(name="acc", bufs=2, space="PSUM") as ACC,
        tc.tile_pool(name="cpsp", bufs=2, space="PSUM") as CPSP,
        tc.tile_pool(name="sps", bufs=2, space="PSUM") as SPS,
    ):
        xp_sb = P1.tile([128, NDH, 2, 2, S], F8, tag="xp")
        wqk_sb = P1.tile([128, HPC, NDH, 2, 2, 128], F8, tag="wqk")
        wv_sb = P1.tile([128, NDH, 2, 2, 256], F8, tag="wv")
        wo01_sb = P1.tile([128, D], F32R, tag="wo01")
        wo2_sb = P1.tile([64, D], F32R, tag="wo2")
        bqk_sb = P1.tile([128, HPC], F32, tag="bqk")
        # q/k transposed per head (separate tiles: matmul operands must share
        # the SBUF base partition, so both live at partitions 0..63)
        qT = [P1.tile([64, S], F32R, tag=f"qT{h}", name=f"qT{h}") for h in range(HPC)]
        kT = [P1.tile([64, S], F32R, tag=f"kT{h}", name=f"kT{h}") for h in range(HPC)]
        if EDGE:
            shift_sb = P1.tile([128, 1], F32, tag="shift")
            # v pairs: [p, key-chunk, i, h*128+c]; i=0: [v8 | ones],
            # i=1: [dv8 | zeros]  (v bias is folded into bo on the host)
            vp = P1.tile([128, NSK, 2, HPC * 128], F8, tag="vp")
        else:
            vp = P1.tile([128, NSK, HPC * 128], F16, tag="vp")

        # PE warmup tile: memset first on Pool so the dummy matmuls can
        # start immediately (HAM clock gate needs sustained PE activity)
        warm = P1.tile([64, 512], F32R, tag="warm")
        nc.gpsimd.memset(warm[:].bitcast(F32), 0.0)

        # DMAs split across the SP and Pool queues: fp16 transfers are small
        # enough that per-DMA overhead dominates, so few big transfers in
        # first-needed order (qk(0,0) needs all of xT sc-chunk 0 + wqk).
        nc.sync.dma_start(wqk_sb[:, 0], wqk[:, 0])
        nc.sync.dma_start(xp_sb[:, 0, :, :, 0:SQ], xp[:, 0, :, :, 0:SQ])
        nc.sync.dma_start(xp_sb[:, 1:NDH, :, :, 0:SQ], xp[:, 1:NDH, :, :, 0:SQ])
        nc.sync.dma_start(bqk_sb[:], bqk[:])
        nc.sync.dma_start(xp_sb[:, :, :, :, SQ:2 * SQ], xp[:, :, :, :, SQ:2 * SQ])
        nc.sync.dma_start(xp_sb[:, :, :, :, 2 * SQ:3 * SQ], xp[:, :, :, :, 2 * SQ:3 * SQ])
        nc.sync.dma_start(xp_sb[:, :, :, :, 3 * SQ:S], xp[:, :, :, :, 3 * SQ:S])
        nc.sync.dma_start(wqk_sb[:, 1:3], wqk[:, 1:3])
        # wv is small and needed early (first v unit)
        nc.gpsimd.dma_start(wv_sb[:], wv[:])
        if EDGE:
            nc.gpsimd.memset(shift_sb[:], SHIFT)
            vpr = vp[:].rearrange("p s i (h m) -> p s i h m", m=128)
            nc.gpsimd.memset(vpr[:, :, 0, :, 64:128], 1.0)
            nc.gpsimd.memset(vpr[:, :, 1, :, 64:128], 0.0)
        else:
            nc.gpsimd.memset(
                vp[:].rearrange("p s (h m) -> p s h m", m=128)[:, :, :, 64:128], 1.0
            )
        nc.gpsimd.dma_start(wo01_sb[:], wo01[:])
        nc.gpsimd.dma_start(wo2_sb[:], wo2[:])

        def qk_unit(h, sc, split_evac=False):
            # one query-chunk of q/k projection for head h (+ bias)
            ps = ACC.tile([128, SQ], F32, tag="acc", name=f"qkps{h}_{sc}")
            chains = ((0, 0), (1, 0), (0, 1))  # (w-residual?, x-residual?)
            for ci, (rw, rx) in enumerate(chains):
                for o in range(NDH):
                    nc.tensor.matmul(
                        ps[:],
                        wqk_sb[:, h, o, :, rw, :],
                        xp_sb[:, o, :, rx, sc * SQ:(sc + 1) * SQ],
                        start=(ci == 0 and o == 0),
                        stop=(ci == 2 and o == NDH - 1),
                        perf_mode=DR,
                    )
            nc.vector.scalar_tensor_tensor(
                qT[h][:, sc * SQ:(sc + 1) * SQ],
                ps[0:64, :],
                1.0 / WSCALE,
                bqk_sb[0:64, h:h + 1].to_broadcast([64, SQ]),
                MULT,
                ADD,
            )
            # partition-shifted copy: psum rows 64..127 -> kT rows 0..63
            # (split for the first unit so the first scores pair can start
            # after the first key chunks land)
            pieces = ((0, 256), (256, SQ)) if split_evac else ((0, SQ),)
            for lo, hi in pieces:
                nc.vector.scalar_tensor_tensor(
                    kT[h][:, sc * SQ + lo:sc * SQ + hi],
                    ps[64:128, lo:hi],
                    1.0 / WSCALE,
                    bqk_sb[64:128, h:h + 1].to_broadcast([64, hi - lo]),
                    MULT,
                    ADD,
                )

        def v_unit(i):
            # two key-chunks (2i, 2i+1) of v = xT.T @ [Wv_h0|Wv_h1|Wv_h2|pad]
            # share one psum tile so the evacuation runs 384 wide (the v bias
            # passes through the softmax average and is folded into bo on
            # the host)
            ps = ACC.tile([128, SQ], F32, tag="acc", name=f"vps{i}")
            for half in range(2):
                mk = 2 * i + half
                reg = ps[:, half * 256:(half + 1) * 256]
                chains = ((0, 0), (1, 0), (0, 1))  # (w-residual?, x-residual?)
                for ci, (rw, rx) in enumerate(chains):
                    for o in range(NDH):
                        nc.tensor.matmul(
                            reg,
                            xp_sb[:, o, :, rx, mk * 128:(mk + 1) * 128],
                            wv_sb[:, o, :, rw, :],
                            start=(ci == 0 and o == 0),
                            stop=(ci == 2 and o == NDH - 1),
                            perf_mode=DR,
                        )
            # hw tensor-scalar ops require <=3D APs: evacuate per key-chunk
            for half in range(2):
                mk = 2 * i + half
                srcc = ps[:, half * 256:half * 256 + HPC * 64].rearrange(
                    "p (h m) -> p h m", m=64)
                if EDGE:
                    dst = vp[:, mk, 0, :].rearrange(
                        "p (h m) -> p h m", m=128)[:, :, 0:64]
                    nc.vector.tensor_scalar(dst, srcc, 1.0 / WSCALE, None, MULT)
                    ddst = vp[:, mk, 1, :].rearrange(
                        "p (h m) -> p h m", m=128)[:, :, 0:64]
                    nc.vector.scalar_tensor_tensor(
                        ddst, srcc, 1.0 / WSCALE, dst, MULT, SUB)
                else:
                    nc.vector.tensor_scalar(
                        vp[:, mk, :].rearrange(
                            "p (h m) -> p h m", m=128)[:, :, 0:64],
                        srcc, 1.0 / WSCALE, None, MULT,
                    )

        def proj_mms(sc, ms, ctx01, ctx2, psA, psB, start2, stop01):
            # the 4 accumulating matmuls of out-chunk (sc, ms); split so the
            # ctx2 half can fire early in the last query-chunk
            def mm2():
                for ps, n0, nw in ((psA, 0, 512), (psB, 512, 256)):
                    nc.tensor.matmul(
                        ps[:],
                        ctx2[:, ms * 128:(ms + 1) * 128],
                        wo2_sb[:, n0:n0 + nw],
                        start=start2, stop=False,
                    )

            def mm01():
                for ps, n0, nw in ((psA, 0, 512), (psB, 512, 256)):
                    nc.tensor.matmul(
                        ps[:],
                        ctx01[:, ms * 128:(ms + 1) * 128],
                        wo01_sb[:, n0:n0 + nw],
                        start=not start2, stop=stop01,
                    )
            return mm2, mm01

        def proj_out(sc, ms, psA, psB, tail=False):
            ot = W2.tile([128, D], BF16, tag="out", name=f"ot{sc}_{ms}")
            # GPSIMD cannot read PSUM on TRN2, so evacuations go to DVE,
            # with ACT (idle after the last exp) taking the wide half of the
            # tail chunks
            if tail:
                nc.scalar.copy(ot[:, 0:512], psA[:])
                nc.vector.tensor_copy(ot[:, 512:768], psB[:])
                deng = (nc.sync, nc.scalar, nc.scalar, nc.sync)[ms % 4]
            else:
                nc.vector.tensor_copy(ot[:, 0:512], psA[:])
                nc.vector.tensor_copy(ot[:, 512:768], psB[:])
                deng = nc.sync if (sc * 4 + ms) % 2 == 0 else nc.gpsimd
            deng.dma_start(
                out[(sc * 4 + ms) * 128:(sc * 4 + ms + 1) * 128, :], ot[:]
            )

        def proj_unit(sc, ms, ctx01, ctx2, tail=False):
            pool = CPSP if (tail and ms % 2 == 1) else ACC
            tag = "cps" if (tail and ms % 2 == 1) else "acc"
            psA = pool.tile([128, 512], F32, tag=tag, name=f"opsA{sc}_{ms}")
            psB = pool.tile([128, 256], F32, tag=tag, name=f"opsB{sc}_{ms}")
            mm2, mm01 = proj_mms(sc, ms, ctx01, ctx2, psA, psB, True, True)
            mm2()
            mm01()
            proj_out(sc, ms, psA, psB, tail)

        filler = deque()

        blocks = []
        for sc in range(NSQ):
            for h in ((2, 0, 1) if sc == NSQ - 1 else (0, 1, 2)):
                blocks.append((sc, h))

        ctxs = {}
        probs_t = {}
        cps_t = {}
        pdt = F8 if EDGE else F16

        def probsv(b, mk):
            sc, h = blocks[b]
            if b not in cps_t:
                cps_t[b] = CPSP.tile([128, SQ], F32, tag="cps", name=f"cps{b}")
            cps, probs = cps_t[b], probs_t[b]
            if EDGE:
                rhs = probs[:, mk * SQ:(mk + 1) * SQ].rearrange(
                    "p (o n) -> p o n", o=1
                ).to_broadcast([128, 2, SQ])
                nc.tensor.matmul(
                    cps[:],
                    vp[:, mk, :, h * 128:(h + 1) * 128],
                    rhs,
                    start=(mk == 0),
                    stop=(mk == NSK - 1),
                    perf_mode=DR,
                )
            else:
                nc.tensor.matmul(
                    cps[:],
                    vp[:, mk, h * 128:(h + 1) * 128],
                    probs[:, mk * SQ:(mk + 1) * SQ],
                    start=(mk == 0),
                    stop=(mk == NSK - 1),
                )

        def finish_block(b, quarters=1):
            # rows 0..63 of cps: unnormalized ctxT; rows 64..127: denominators
            # (the last block normalizes in quarters so each out-proj chunk
            # can start as soon as its ctx columns are ready)
            sc, h = blocks[b]
            cps = cps_t.pop(b)
            ctx01, ctx2 = ctxs[sc]
            r = W2.tile([64, SQ], F32, tag="recip", name=f"r{b}")
            dst = ctx01[h * 64:(h + 1) * 64, :] if h < 2 else ctx2[:]
            w = SQ // quarters
            for q in range(quarters):
                sl = slice(q * w, (q + 1) * w)
                nc.vector.reciprocal(r[:, sl], cps[64:128, sl])
                nc.vector.tensor_tensor(dst[:, sl], cps[0:64, sl], r[:, sl], MULT)

        def emit_pair(b, j):
            # scores for key chunks (2j, 2j+1) + 1024-wide exp
            sc, h = blocks[b]
            probs = probs_t[b]
            sps = SPS.tile([128, 2 * SQ], F32, tag="sps", name=f"sps{b}_{j}")
            for half in range(2):
                mk = 2 * j + half
                nc.tensor.matmul(
                    sps[:, half * SQ:(half + 1) * SQ],
                    kT[h][:, mk * 128:(mk + 1) * 128],
                    qT[h][:, sc * SQ:(sc + 1) * SQ],
                    start=True,
                    stop=True,
                )
            # probs = exp(scores / sqrt(DH) + SHIFT); no max-subtraction
            # needed (scores/8 ~ N(0,1): the shift keeps fp8 in range)
            if EDGE:
                nc.scalar.activation(
                    probs[:, j * 2 * SQ:(j + 1) * 2 * SQ], sps[:], EXP,
                    bias=shift_sb[:], scale=0.125,
                )
            else:
                nc.scalar.activation(
                    probs[:, j * 2 * SQ:(j + 1) * 2 * SQ], sps[:], EXP,
                    scale=0.125,
                )

        # PE warmup: the HAM clock gate needs ~3-4us of sustained activity
        # to release full clock; burn the initial DMA wait on dummy matmuls.
        wps = ACC.tile([128, 512], F32, tag="acc", name="warmps")
        for _ in range(7):
            nc.tensor.matmul(wps[:], warm[:, 0:128], warm[:], start=True, stop=True)
        # pre-load the ACT exp table set during the same dead time
        wact = P1.tile([64, 1], F16, tag="wact")
        nc.scalar.activation(wact[:], warm[:, 0:1].bitcast(F32), EXP, scale=0.125)

        qk_unit(0, 0)
        # phase-A fillers, ordered to meet their consumers' deadlines:
        # kT[0] chunks for block 0's later pairs, v-pairs for block 0's
        # probs@V (consumed one block later), qk head 1/2 for blocks 1/2
        filler.extend([
            lambda: qk_unit(0, 1), lambda: v_unit(0),
            lambda: qk_unit(0, 2), lambda: qk_unit(0, 3),
            lambda: v_unit(1), lambda: qk_unit(1, 0),
            lambda: v_unit(2), lambda: v_unit(3),
            lambda: qk_unit(1, 1), lambda: v_unit(4),
            lambda: qk_unit(1, 2), lambda: v_unit(5),
            lambda: qk_unit(1, 3), lambda: v_unit(6),
            lambda: v_unit(7), lambda: qk_unit(2, 0),
            lambda: qk_unit(2, 1), lambda: qk_unit(2, 2), lambda: qk_unit(2, 3),
        ])
        # (pops_per_j, pop_stride) per block; out-proj fillers are appended
        # once their ctx completes (two blocks after the last head's block)
        popsched = {0: (2, 1), 1: (1, 1), 2: (1, 2),
                    4: (1, 3), 5: (1, 3), 7: (1, 3), 8: (1, 3),
                    10: (1, 4), 11: (1, 3)}

        last_ps = {}

        def early_ms0():
            psA = ACC.tile([128, 512], F32, tag="acc", name="lpsA")
            psB = ACC.tile([128, 256], F32, tag="acc", name="lpsB")
            last_ps[0] = (psA, psB)
            proj_mms(NSQ - 1, 0, *ctxs[NSQ - 1], psA, psB, True, True)[0]()

        NB = len(blocks)
        for b in range(NB):
            sc, h = blocks[b]
            if sc not in ctxs:
                ctxs[sc] = (
                    W2.tile([128, SQ], F32R, tag="ctx01", name=f"c01_{sc}"),
                    W2.tile([64, SQ], F32R, tag="ctx2", name=f"c2_{sc}"),
                )
            probs_t[b] = PR.tile([128, NSK * SQ], pdt, tag="probs", name=f"pr{b}")
            pops, stride = popsched.get(b, (0, 1))
            for j in range(NSK // 2):
                emit_pair(b, j)
                if pops and j % stride == 0:
                    for _ in range(pops):
                        if filler:
                            filler.popleft()()
                if 1 <= b <= NB - 3:
                    # probs@V runs one full block behind scores/exp so the
                    # activation engine is never paced by the v/qk fillers
                    probsv(b - 1, 2 * j)
                    probsv(b - 1, 2 * j + 1)
                elif b == NB - 2:
                    # second-to-last block also runs its own probs@V nearly
                    # in-line so the last block (and the tail) stay light
                    probsv(b - 1, 2 * j)
                    probsv(b - 1, 2 * j + 1)
                    if j > 0:
                        probsv(b, 2 * j - 2)
                        probsv(b, 2 * j - 1)
                elif b == NB - 1:
                    if j == 0:
                        probsv(NB - 2, NSK - 2)
                        probsv(NB - 2, NSK - 1)
                    if j == 1:
                        finish_block(NB - 2)
                    if j > 0:
                        probsv(NB - 1, 2 * j - 2)
                        probsv(NB - 1, 2 * j - 1)
            if 1 <= b <= NB - 2:
                finish_block(b - 1)
            if b == 3:
                for ms in range(4):
                    filler.append(lambda ms=ms: proj_unit(0, ms, *ctxs[0]))
            elif b == 6:
                for ms in range(4):
                    filler.append(lambda ms=ms: proj_unit(1, ms, *ctxs[1]))
            elif b == 9:
                for ms in range(4):
                    filler.append(lambda ms=ms: proj_unit(2, ms, *ctxs[2]))
                filler.append(early_ms0)
        for mk in range(NSK - 2, NSK):
            probsv(NB - 1, mk)
        finish_block(NB - 1, quarters=1)
        # tail: the ms0 psum already holds the ctx2 half; evacs alternate
        # DVE/Pool to shorten the drain
        sc = NSQ - 1
        ctx01, ctx2 = ctxs[sc]
        for ms in range(SQ // 128):
            if ms in last_ps:
                psA, psB = last_ps[ms]
                proj_mms(sc, ms, ctx01, ctx2, psA, psB, True, True)[1]()
                proj_out(sc, ms, psA, psB, tail=True)
            else:
                proj_unit(sc, ms, ctx01, ctx2, tail=True)
        while filler:
            filler.popleft()()


_CACHE = {}


def _get_module():
    if "nc" not in _CACHE:
        _CACHE["nc"] = _build_module()
    return _CACHE["nc"]


def make_in_maps(x, Wq, Wk, Wv, bq, bk, bv, Wo):
    # bv is folded into the output bias on the host (the v bias passes
    # through the softmax average untouched): see kernel().
    f = np.float32
    f8 = mybir.dt.np(F8)

    def pair(a):
        # fp8 value + residual along a new axis: a ~ a8 + da8 to ~0.1%
        a8 = a.astype(f8)
        da8 = (a - a8.astype(f)).astype(f8)
        return np.stack([a8, da8], axis=-2 if a.ndim == 4 else 0)

    in_maps = []
    for c in range(NCORES):
        b = c // CORES_PER_BATCH
        hh = [HPC * (c % CORES_PER_BATCH) + i for i in range(HPC)]
        # xp [p, o, i, r, s]: row 256*o + 128*i + p of x[b].T, r = val/resid
        xt = np.ascontiguousarray(
            x[b].T.reshape(NDH, 2, 128, S).transpose(2, 0, 1, 3))
        x8 = xt.astype(f8)
        dx8 = (xt - x8.astype(f)).astype(f8)
        xp = np.stack([x8, dx8], axis=3)
        # wqk [p, h, o, i, r, m]
        wqk = np.stack(
            [np.concatenate([Wq[h], Wk[h]], axis=1) for h in hh]
        )  # [3, 768, 128]
        wqk = wqk.reshape(HPC, NDH, 2, 128, 128).transpose(3, 0, 1, 2, 4)
        wqk = wqk * np.float32(WSCALE)
        w8 = wqk.astype(f8)
        dw8 = (wqk - w8.astype(f)).astype(f8)
        wqk8 = np.stack([w8, dw8], axis=4)
        # wv [p, o, i, r, n]
        wv_stack = np.concatenate(
            [Wv[h] for h in hh] + [np.zeros((D, 64), f)], axis=1
        )  # [768, 256]
        wv_stack = wv_stack.reshape(NDH, 2, 128, 256).transpose(2, 0, 1, 3)
        wv_stack = wv_stack * np.float32(WSCALE)
        v8 = wv_stack.astype(f8)
        dv8 = (wv_stack - v8.astype(f)).astype(f8)
        wv8 = np.stack([v8, dv8], axis=3)
        in_maps.append({
            "xp": np.ascontiguousarray(xp),
            "wqk": np.ascontiguousarray(wqk8),
            "wv": np.ascontiguousarray(wv8),
            "wo01": np.ascontiguousarray(Wo[hh[0] * DH:(hh[0] + 2) * DH, :]).astype(f, copy=False),
            "wo2": np.ascontiguousarray(Wo[hh[2] * DH:(hh[2] + 1) * DH, :]).astype(f, copy=False),
            "bqk": np.ascontiguousarray(
                np.stack([np.concatenate([bq[h], bk[h]]) for h in hh], axis=1)
            ).astype(f, copy=False),
        })
    return in_maps


def gather(results, bo):
    out = np.empty((B, S, D), np.float32)
    for b in range(B):
        acc = results[b * CORES_PER_BATCH]["out"].astype(np.float32, copy=True)
        for c in range(b * CORES_PER_BATCH + 1, (b + 1) * CORES_PER_BATCH):
            acc += results[c]["out"]
        out[b] = acc + bo[None, :].astype(np.float32)
    return out


def kernel(x, Wq, Wk, Wv, bq, bk, bv, Wo, bo, c=0, **_unused):
    x, Wq, Wk, Wv, bq, bk, bv, Wo, bo = (
        np.asarray(a, np.float32) for a in (x, Wq, Wk, Wv, bq, bk, bv, Wo, bo)
    )
    nc = _get_module()
    in_maps = make_in_maps(x, Wq, Wk, Wv, bq, bk, bv, Wo)
    res = run_bass_kernel_spmd(nc, in_maps, list(range(NCORES)))
    # ctx = sum_t p_t (v_t + bv) / sum_t p_t = ctx' + bv, so bv rides into
    # the output bias: bo_eff = bo + concat_h(bv_h) @ Wo
    bo_eff = (bo.astype(np.float64) +
              bv.reshape(H * DH).astype(np.float64) @ Wo.astype(np.float64)
              ).astype(np.float32)
    return gather(res.results, bo_eff)
